# revision 73
# baseline (speedup 1.0000x reference)
"""Trainium2 Bass kernel for nn_BilinAndFwdComboVecComp.

Math (B=8, S=256, C=256, V=64):
  final[b,s,z,k] = tanh( sum_ij ctx[b,s,i] ctx[b,z,j] W'[i,j,k] + A[b,z,k] + Bt[b,s,k] )
where
  W'[i,j,k] = W[i,j,k] + (i==j) * linmul_w[k,i]          (folds the `mul` branch)
  A[b,z,k]  = ctx[b] @ (lin1_w+lindiff_w).T + (lin1_b + bias + linmul_b
                                               + lindiff_b + lin2_b)
  Bt[b,s,k] = ctx[b] @ (lin2_w-lindiff_w).T              (sans bias; in A's const)

Sharding: V split across the 8 cores (8 k-values per core). Each core:
  phase 1: tmp2[i,(k,z)] = sum_j Wt[j,(k,i)]^T @ ctxT[j,z]   (W-stationary;
           PSUM drained by DVE/ACT copies in parallel)
  phase 2: out[s,(k,z)]  = ctxT[:,s]^T @ tmp2[:,(k,z)], tanh, DMA to a
           (B,S,KV,S) scratch; host transposes/concats.

vs the previous 90.6us version (~86.5us now): the 64 fold matmuls (A/Bt/
bias add via a zero-padded 18-row contraction; 20% of PE work) are
restructured:
  - Bt[s,k] = sum_i ctx[s,i] L2d[k,i] folds into phase 2's dense contraction
    by adding L2d[k,i] to tmp2[i,(k,z)] during the phase-1 PSUM drain
    (tensor_scalar_add / activation-bias: per-partition scalar, free).
  - A[b,(k,z)] for 5 batches: psum-init matmuls (ones2 zero-padded to K=128
    so they stream at the full ~216ns rate; short-K pays +210ns). For the
    other 3 (spread out: {1,3,6}): a DRAM stride-0-partition broadcast DMA
    to a [128,2048] SBUF tile + a DVE tensor_tensor add (1.22us/tile, f16
    out) - cheaper than PE when DVE has local slack.
PE stream: 296 N=512 fp16 matmuls at ~216-222ns warm (vs 320 before).
Empirical engine rates baked into the schedule: DVE/ACT ~0.7-1.2ns/col with
~230-350ns/op overhead; ACT tanh 687ns/[128,1024] from PSUM vs 1148 from
SBUF (so PE-init batches tanh straight from psum); sync-engine DMA issue
~0.7us/instruction (critical loads get the sync ring exclusively, merged
into one DMA per ctx pair / wt window via host repacking into 1-2KB
contiguous lines); SWDGE (gpsimd) issue cadence throttles bulk loads
naturally. 7 warmup matmuls bridge engine-init + input-DMA latency so the
HAM clock is at K=8/8 when real work starts (any >2us PE idle drops it to
K=4 for ~7us). Stores are 256KB per psum tile striped over the sync+scalar
HWDGE rings (SWDGE store drains stretch the tail); the final tile is split
in half across both rings.
"""

import numpy as np

B, S, C, V = 8, 256, 256, 64
NCORES = 8
KV = V // NCORES  # k-values per core
N_WARM = 10       # warmup matmuls on a zero tile: must fully bridge to
                  # input-ready (~12us) - a PE idle right at the clock-ramp
                  # k6->k8 transition delays full clock by several us
# Batches whose A-add runs on the PE as psum-init matmuls (ones2[128,128]
# zero-padded lhsT (x) A-rows, K=128 so they stream at the full 216ns rate;
# short-K matmuls pay +210ns) instead of a DVE tensor_tensor + ACT
# tanh-from-SBUF (1218 + 1148ns/tile vs 687ns tanh-from-PSUM).
# DVE-add batches {1,3,4,6} are spread out (never adjacent) so the DVE's
# 2.43us-per-batch-sc add cost overlaps phase1/PE-init stretches. b6's two
# sc1 tiles are PE-init anyway so b7's first psum tiles (2-buf rotation)
# aren't gated on a DVE add (see phase2).
PE_BATCHES = (0, 2, 4, 5, 7)


def _host_prep(ctx, W, bias, lin1_w, lin1_b, lin2_w, lin2_b,
               linmul_w, linmul_b, lindiff_w, lindiff_b):
    f = np.float32
    ctx = np.asarray(ctx, f)
    Wp = np.array(W, f)
    Wp[np.arange(C), np.arange(C), :] += np.asarray(linmul_w, f).T
    Wt = Wp.transpose(1, 0, 2)  # [j, i, k]

    A = ctx @ (np.asarray(lin1_w, f) + np.asarray(lindiff_w, f)).T \
        + (np.asarray(lin1_b, f) + np.asarray(bias, f) + np.asarray(linmul_b, f)
           + np.asarray(lindiff_b, f) + np.asarray(lin2_b, f))
    L2d = np.asarray(lin2_w, f) - np.asarray(lindiff_w, f)  # [V, C]

    # ctx packed in the phase-1 SBUF tile layout: [pair, c, jchunk, h, z]
    # (h = which batch of the pair): one 2KB-contiguous DMA line per
    # partition -> a single DMA per pair
    ctxT = ctx.transpose(0, 2, 1)  # [B, C, S]
    ctxp = np.ascontiguousarray(
        ctxT.reshape(B // 2, 2, 2, 128, S)      # [pair, h, j, c, z]
            .transpose(0, 3, 2, 1, 4)           # [pair, c, j, h, z]
    ).astype(np.float16)

    # window-major wt packing: [p, (w, j, c in w)] so every window DMA is one
    # contiguous 1-2KB line per partition (128 lines vs 256 x 512B)
    wins = [(0, 256), (256, 768), (768, 1280), (1280, 1792), (1792, 2048)]

    per_core = []
    for c in range(NCORES):
        ks = slice(c * KV, (c + 1) * KV)
        # wt layout: [j*128+p, kk*C + i] -> window-major packed [p, 4096]
        wt0 = Wt[:, :, ks].transpose(0, 2, 1).reshape(2, 128, KV * C)
        wt = np.ascontiguousarray(np.concatenate(
            [np.concatenate([wt0[0][:, lo:hi], wt0[1][:, lo:hi]], axis=1)
             for lo, hi in wins], axis=1)).astype(np.float16)
        # A in (k, z) layout per batch, f16 hi + residual lo rows (exact to
        # ~2^-22; consumed by the K=128 zero-padded psum-init matmuls)
        a_f32 = np.ascontiguousarray(
            A[:, :, ks].transpose(0, 2, 1).reshape(B, KV * S))
        a_hi = a_f32.astype(np.float16)
        a_lo = (a_f32 - a_hi.astype(np.float32)).astype(np.float16)
        a_c = np.stack([a_hi, a_lo], axis=1)  # [B, 2, KV*S]
        # L2d slice transposed: [i, k] split into two 128-row i-chunks
        l2dT = np.ascontiguousarray(L2d[ks].T.reshape(2, 128, KV))
        per_core.append({"ctxp": ctxp, "wt": wt, "a": a_c, "l2dT": l2dT,
                         "ones2": np.ones((2, 128), np.float16)})
    return per_core


def _build_program():
    import concourse.tile as tile
    import concourse.mybir as mybir
    from concourse import bacc
    from contextlib import ExitStack

    f32 = mybir.dt.float32
    f16 = mybir.dt.float16
    TANH = mybir.ActivationFunctionType.Tanh

    nc = bacc.Bacc("TRN2", target_bir_lowering=False, debug=False)
    ctxp_d = nc.dram_tensor("ctxp", [B // 2, 128, 2 * 2 * S], f16, kind="ExternalInput").ap()
    wt_d = nc.dram_tensor("wt", [128, 2 * KV * C], f16, kind="ExternalInput").ap()
    a_d = nc.dram_tensor("a", [B, 2, KV * S], f16, kind="ExternalInput").ap()
    ones_d = nc.dram_tensor("ones2", [2, 128], f16, kind="ExternalInput").ap()
    l2d_d = nc.dram_tensor("l2dT", [2, 128, KV], f32, kind="ExternalInput").ap()
    # out scratch is (k, z)-ordered; the host transposes back to (z, k)
    out_d = nc.dram_tensor("out", [B, S, KV, S], f16, kind="ExternalOutput").ap()

    with tile.TileContext(nc) as tc, ExitStack() as es:
        ctx_pool = es.enter_context(tc.tile_pool(name="ctxp", bufs=8))
        wt_pool = es.enter_context(tc.tile_pool(name="wtp", bufs=2))
        l2d_pool = es.enter_context(tc.tile_pool(name="l2dp", bufs=1))
        arep_pool = es.enter_context(tc.tile_pool(name="arep", bufs=8))
        tmp2_pool = es.enter_context(tc.tile_pool(name="tmp2p", bufs=8))
        ot_pool = es.enter_context(tc.tile_pool(name="otp", bufs=4))
        ot2_pool = es.enter_context(tc.tile_pool(name="ot2p", bufs=6))

        # warmup is emitted first so its wsrc memset leads the DVE queue —
        # the PE can start ramping the HAM clock at engine-init time
        def warmup(ps2_pool):
            wsrc = es.enter_context(tc.tile_pool(name="warmp", bufs=1)).tile(
                [128, 512], f16, name="wsrc", bufs=1)
            nc.vector.memset(wsrc[:], 0.0)
            # zero-padded ones lhsT for the A psum-init matmuls: rows 0/1 = 1
            # (hi/lo A rows), rows 2..127 = 0 so K=128 streams at full rate
            ones = es.enter_context(tc.tile_pool(name="onesp", bufs=1)).tile(
                [128, 128], f16, name="ones2", bufs=1)
            nc.vector.memset(ones[:], 0.0)
            nc.gpsimd.dma_start(ones[0:2, :], ones_d[:])
            wps = ps2_pool.tile([128, 1024], f32, name="ps2")
            for i in range(N_WARM):
                nc.tensor.matmul(wps[:, (i % 2) * 512:(i % 2) * 512 + 512],
                                 wsrc[:, 0:128], wsrc[:], start=True, stop=True)
            # preload the tanh spline tables while the PE warms up, so the
            # ~1.5us ACT_TABLE_LOAD doesn't stall the first real tanh
            tt = ot2_pool.tile([128, 8], f16, name="ttl", bufs=1)
            nc.scalar.activation(tt[:], wsrc[:, 0:8], TANH)
            return ones

        # Input staging. The sync-engine DMA issue cost is ~0.7us per
        # instruction, so the critical path (ctx pair 0 + progressive wt
        # column windows, consumed kk-major by phase 1) gets the sync ring
        # EXCLUSIVELY, with everything merged into one DMA per pair/window
        # (2KB lines). All bulk/small loads + the arep broadcasts ride the
        # gpsimd ring, queued in need-order.
        ctxp_sb = {}

        def load_ctx_pair(p, eng):
            t = ctx_pool.tile([128, 4 * S], f16, name=f"ctx_{p}", bufs=1)
            eng.dma_start(t[:], ctxp_d[p])
            ctxp_sb[p] = t

        # window-major packed wt: sb col of (j, c) = 2*cum[w] + j*len[w]
        # + (c - cum[w]) where w is the window containing c
        wcum = [0, 256, 768, 1280, 1792, 2048]

        def wt_col(j, c):
            w = next(i for i in range(5) if wcum[i] <= c < wcum[i + 1])
            return 2 * wcum[w] + j * (wcum[w + 1] - wcum[w]) + (c - wcum[w])

        def load_inputs():
            load_ctx_pair(0, nc.sync)
            wt_sb = wt_pool.tile([128, 2 * KV * C], f16, name="wt", bufs=1)
            for w in range(5):
                lo, hi = 2 * wcum[w], 2 * wcum[w + 1]
                nc.sync.dma_start(wt_sb[:, lo:hi], wt_d[:, lo:hi])
            # small loads on the gpsimd ring, in need-order:
            load_ctx_pair(1, nc.gpsimd)
            # L2d bias columns: needed by the first PSUM drain (~11us)
            l2d_sb = l2d_pool.tile([128, 2 * KV], f32, name="l2dT", bufs=1)
            nc.gpsimd.dma_start(
                l2d_sb[:].rearrange("p (ch k) -> p ch k", ch=2),
                l2d_d.rearrange("ch p k -> p ch k"))
            # 3 rotating A slots inside one persistent tile for the psum-init
            # matmul rhs: rows 0/1 carry A hi/lo for batch b (slot b%3, 8KB
            # DMA'd per batch inside phase2), rows 2..127 are zeroed ONCE
            # here and never rewritten
            az = arep_pool.tile([128, 3 * KV * S], f16, name="az", bufs=1)
            for sl in range(3):
                nc.gpsimd.memset(az[:, sl * KV * S:(sl + 1) * KV * S], 0.0)
            # bulk transfers (ctx pairs 2/3, A broadcasts) also ride the
            # gpsimd ring: its ~0.65us per-DMA issue cadence throttles them
            # naturally, so they trickle in behind the critical sync-ring
            # loads; none of them is needed before ~30us.
            load_ctx_pair(2, nc.gpsimd)
            load_ctx_pair(3, nc.gpsimd)
            # A[b] broadcast to all 128 partitions straight from DRAM
            # (stride-0 partition AP) for the DVE-add batches; 512KB each.
            arep = {}
            for b in range(B):
                if b in PE_BATCHES:
                    continue
                t = arep_pool.tile([128, KV * S], f16, name=f"arep_{b}", bufs=1)
                nc.gpsimd.dma_start(t[:], a_d[b:b + 1, 0].to_broadcast([128, KV * S]))
                arep[b] = t
            return wt_sb, l2d_sb, az, arep

        tmp2p = {}

        def phase1(pg, ps1_pool, copy_engines=("vector",), chs=(0, 1)):
            # kk-major so the wt columns are consumed left-to-right, matching
            # the progressive wt window DMAs
            ce = [0]
            for ch in chs:
                for p in pg:
                    tmp2p[p, ch] = tmp2_pool.tile([128, 2 * KV * S], f16, name="tmp2")
            for kk in range(KV):
                for ch in chs:  # i-chunk (output partition of tmp2)
                    ps = {}
                    for p in pg:
                        ps[p] = ps1_pool.tile([128, 2 * S], f32, name="ps1")
                    for j in range(2):  # contraction chunk
                        off = wt_col(j, kk * C + ch * 128)
                        lhsT = wt_sb[:, off: off + 128]
                        for p in pg:
                            nc.tensor.matmul(
                                ps[p][:], lhsT,
                                ctxp_sb[p][:, j * 2 * S:(j + 1) * 2 * S],
                                start=(j == 0), stop=(j == 1),
                            )
                    bias_ap = l2d_sb[:, ch * KV + kk: ch * KV + kk + 1]
                    for p in pg:
                        # drain + fold Bt: tmp2[i,(h,kk,z)] = psum + L2d[kk,i]
                        dst = tmp2p[p, ch][:].rearrange("q (h k z) -> q h k z", h=2, k=KV)
                        src_ap = ps[p][:].rearrange("q (h z) -> q h z", h=2)
                        eng = copy_engines[ce[0] % len(copy_engines)]
                        ce[0] += 1
                        if eng == "vector":
                            nc.vector.tensor_scalar_add(dst[:, :, kk, :], src_ap, bias_ap)
                        else:
                            nc.scalar.add(dst[:, :, kk, :], src_ap, bias_ap)

        st_ctr = [0]

        def phase2(bg, ps2_pool, split_store=False):
            for b in bg:
                pe_init_b = b in PE_BATCHES
                sl_off = (b % 3) * KV * S
                if pe_init_b or b == 6:
                    # stage this batch's A hi/lo rows into its rotating slot
                    nc.gpsimd.dma_start(az[0:2, sl_off:sl_off + KV * S], a_d[b])
                for sc in range(2):
                    hoff = (b % 2) * KV * S
                    for t in range(2):  # double-bank psum tiles, 2 n-chunks each
                        # b6's last two tiles are PE-init so b7's first
                        # psum tiles (2-buf rotation) aren't gated on a DVE
                        # add
                        pe_init = pe_init_b or (b == 6 and sc == 1)
                        pst = ps2_pool.tile([128, 1024], f32, name="ps2")
                        n0 = 2 * t
                        if pe_init:
                            # psum = A broadcast over the s partitions
                            # (zero-padded ones2[128,128] (x) A rows, K=128
                            # full-rate matmuls; one per psum bank)
                            for n in (n0, n0 + 1):
                                nc.tensor.matmul(
                                    pst[:, (n % 2) * 512:(n % 2) * 512 + 512],
                                    ones_sb[:],
                                    az[:, sl_off + n * 512:sl_off + (n + 1) * 512],
                                    start=True, stop=False)
                        for st in range(2):  # contraction chunk; one LDW per 2 MMs
                            soff = st * 2 * S + (b % 2) * S + sc * 128
                            lhsT = ctxp_sb[b // 2][:, soff: soff + 128]
                            for n in (n0, n0 + 1):
                                nc.tensor.matmul(
                                    pst[:, (n % 2) * 512:(n % 2) * 512 + 512], lhsT,
                                    tmp2p[b // 2, st][:, hoff + n * 512:hoff + (n + 1) * 512],
                                    start=False if pe_init else (st == 0),
                                    stop=(st == 1),
                                )
                        if pe_init:
                            tanh_src = pst[:]
                        else:
                            # DVE: psum + A (broadcast tile) -> f16 staging
                            ot = ot_pool.tile([128, 1024], f16, name="ot")
                            nc.vector.tensor_add(ot[:], pst[:],
                                                 arep[b][:, t * 1024:(t + 1) * 1024])
                            tanh_src = ot[:]
                        ot2 = ot2_pool.tile([128, 1024], f16, name="ot2")
                        if split_store and sc == 1 and t == 1:
                            # very last tile: split tanh + store per psum bank so
                            # each half-store starts as soon as its half is done,
                            # halved across two queues for the shortest drain
                            # (sync + scalar; the gpsimd queue tends to carry
                            # a backlog at this point)
                            for hd in range(2):
                                nc.scalar.activation(ot2[:, hd * 512:(hd + 1) * 512],
                                                     tanh_src[:, hd * 512:(hd + 1) * 512],
                                                     TANH)
                                eng = nc.sync if hd == 0 else nc.scalar
                                eng.dma_start(
                                    out_d[b, sc * 128:(sc + 1) * 128,
                                          4 + 2 * hd:6 + 2 * hd]
                                    .rearrange("s k z -> s (k z)"),
                                    ot2[:, hd * 512:(hd + 1) * 512],
                                )
                        else:
                            nc.scalar.activation(ot2[:], tanh_src, TANH)
                            # mid-kernel stores stripe over 3 rings (the
                            # gpsimd engine is idle and its 0.65us issue cost
                            # is free; every scalar-ring store costs the ACT
                            # engine ~0.6us on top of its tanh stream); the
                            # last ~quarter go HWDGE-only so the end-of-
                            # kernel queue drain stays short
                            if st_ctr[0] < 22:
                                eng = (nc.sync, nc.gpsimd, nc.scalar)[st_ctr[0] % 3]
                            else:
                                eng = (nc.sync, nc.scalar)[st_ctr[0] % 2]
                            st_ctr[0] += 1
                            eng.dma_start(
                                out_d[b, sc * 128:(sc + 1) * 128, 4 * t:4 * t + 4]
                                .rearrange("s k z -> s (k z)"),
                                ot2[:],
                            )

        ps1_pool = es.enter_context(tc.tile_pool(name="ps1", bufs=4, space="PSUM"))
        ps2_pool = es.enter_context(tc.tile_pool(name="ps2", bufs=2, space="PSUM"))
        # drains ~2:1 on DVE (ACT carries the tanh stream)
        mix = ("vector", "vector", "scalar")
        ones_sb = warmup(ps2_pool)
        wt_sb, l2d_sb, az, arep = load_inputs()
        phase1([0], ps1_pool, copy_engines=mix)
        phase2([0], ps2_pool)
        phase1([1], ps1_pool, copy_engines=mix)
        phase2([1], ps2_pool)
        phase2([2], ps2_pool)
        phase1([2], ps1_pool, copy_engines=mix)
        phase2([3], ps2_pool)
        phase1([3], ps1_pool, copy_engines=("vector", "scalar"), chs=(0,))
        phase2([4], ps2_pool)
        phase1([3], ps1_pool, copy_engines=("vector", "scalar"), chs=(1,))
        phase2([5], ps2_pool)
        phase2([6], ps2_pool)
        phase2([7], ps2_pool, split_store=True)

    nc.compile()
    return nc


def _install_profile_hook():
    """Register the NTFF profile hook that the image's boot skipped
    (antenv.axon_hooks shim is missing in this container)."""
    import sys as _sys
    import types as _types
    try:
        import antenv
        if "antenv.axon_hooks" not in _sys.modules:
            m = _types.ModuleType("antenv.axon_hooks")
            _h = [None]
            m.set_axon_ntff_profile_hook = lambda h: _h.__setitem__(0, h)
            m.get_axon_ntff_profile_hook = lambda: _h[0]
            _sys.modules["antenv.axon_hooks"] = m
            antenv.axon_hooks = m
        from antenv.axon_hooks import set_axon_ntff_profile_hook, get_axon_ntff_profile_hook
        if get_axon_ntff_profile_hook() is None:
            from trn_agent_boot.trn_boot import _ntff_profile_via_ctypes
            set_axon_ntff_profile_hook(_ntff_profile_via_ctypes("/opt/axon/libaxon_pjrt.so"))
    except Exception:
        pass


def run(inputs, trace=False, repeats=1):
    """Returns (full_output, BassKernelResults)."""
    from concourse.bass_utils import run_bass_kernel_spmd

    if trace:
        _install_profile_hook()
    per_core = _host_prep(**inputs)
    nc = _build_program()
    import os as _os
    _tc = [int(x) for x in _os.environ.get("KERNEL_TRACE_CORES", "0").split(",")]
    times = []
    for r in range(repeats):
        res = run_bass_kernel_spmd(nc, per_core, list(range(NCORES)), trace=trace,
                                   trace_cores=_tc if trace else None)
        if res.exec_time_ns is not None:
            times.append(res.exec_time_ns)
    if times:
        res.all_exec_times_ns = times
    # per-core scratch is (B, S, KV, S) with k-major planes: swap to (B,S,S,KV)
    out = np.concatenate(
        [res.results[c]["out"].astype(np.float32).transpose(0, 1, 3, 2)
         for c in range(NCORES)], axis=3)
    out = np.ascontiguousarray(out)
    return out, res


def kernel(**inputs) -> np.ndarray:
    out, _ = run(inputs, trace=False)
    return out
